# revision 52
# baseline (speedup 1.0000x reference)
"""MoE layer (routed top-2 experts + shared SwiGLU expert) on 8 TRN2 NeuronCores.

Sharding strategy (per spec hint):
  - Routed experts: expert-parallel. Core e holds W1/W2/W3[e]; the host computes
    the router (bit-matching the reference's jax fp32 computation on CPU), gathers
    each expert's assigned tokens (top-2 of 8 per token => ~T/4 tokens per expert),
    and ships a [C, D] token block per core (C = max expert count). This is exact
    vs. the dense reference since w_full is zero for non-selected experts.
  - Shared expert: data-parallel on tokens. Core e processes tokens
    [e*T/8, (e+1)*T/8) through the full shared SwiGLU (weights replicated).
  - Combine: host scatter-add of weighted routed outputs + shared outputs.

Device kernel per core: two SwiGLU FFN passes (shared block, routed block):
    hT = (W1^T x^T) [H, Ttok]  (PSUM f32, accumulated over D/128 chunks)
    h2T = hT * silu(h3T)       (ACT sigmoid + DVE muls, cast to bf16)
    yT = W2^T @ h2T            [D, Ttok]  (token dim moving => ragged counts
                               cost proportionally; output rows contiguous)
All matmuls in bf16 with fp32 PSUM accumulation; outputs written bf16.

All inputs are host-packed p-major ([128, k, cols] matching the SBUF tile
layout) so each tensor loads with one large contiguous-per-partition DMA:
the lead-in to the first real matmul is ~4us instead of ~10us of small
descriptor-limited transfers.
"""

from contextlib import ExitStack

import numpy as np
import ml_dtypes

import concourse.bacc as bacc
import concourse.tile as tile
from concourse import mybir
from concourse.bass_utils import run_bass_kernel_spmd

# Problem constants (hardcoded per the self-contained-kernel contract)
B, S, D, H, E, TOPK = 2, 2048, 1024, 2048, 8, 2
SCALE = 1.0 / float(np.sqrt(D))
NCORES = 8
P = 128
KD = D // P     # contraction chunks for phase A (8)
KH = H // P     # h tiles / phase-B contraction chunks (16)
BF16 = ml_dtypes.bfloat16

# test.py introspection: last BassKernelResults (exec_time_ns when BASS_TRACE=1)
LAST_RESULTS = None

_NC_CACHE = {}

WARMUP_MM = 44    # ~4.5us of N=128 matmuls: trips the HAM clock-gate to 8/8
                  # and keeps the PE busy until the first input DMAs land
                  # (an idle gap >3.4us would re-throttle the clock).
LEADW = 128       # pass-1 lead piece: shipped as its own host-packed
                  # contiguous tensor (2KB/partition lines), so it can be one
                  # h-tile wide without tripping the 512B descriptor floor —
                  # the first matmul then needs only x + 2*0.25MB of weights.
# Remaining weight-column DMA piece boundaries for pass 1: sized so later
# h-tiles stay ahead of the PE's ~3.4us/tile consumption; every slice
# segment is >=768B. Pass 2 is WAR-gated on pass 1 anyway so it uses coarse
# pieces (fewer issues).
PIECES_FINE = (512, 1280, H)
PIECES_COARSE = (512, H)


def _ensure_ntff_hook():
    """run_bass_kernel_spmd(trace=True) imports antenv.axon_hooks, which this
    image's antenv lacks. Install a stub (wired to the ctypes NTFF profiler if
    available) so a BASS_TRACE=1 environment doesn't crash the kernel."""
    import sys
    import types

    try:
        import antenv.axon_hooks  # noqa: F401

        return
    except ImportError:
        pass
    try:
        import antenv
    except ImportError:
        return
    mod = types.ModuleType("antenv.axon_hooks")
    holder = [None]
    mod.set_axon_ntff_profile_hook = lambda h: holder.__setitem__(0, h)
    mod.get_axon_ntff_profile_hook = lambda: holder[0]
    sys.modules["antenv.axon_hooks"] = mod
    antenv.axon_hooks = mod
    try:
        import trn_agent_boot.trn_boot as tb

        mod.set_axon_ntff_profile_hook(
            tb._ntff_profile_via_ctypes("/opt/axon/libaxon_pjrt.so")
        )
    except Exception:
        pass
    # In hook-less images the artifact share upload is likely unavailable too;
    # make the trace path's upload best-effort instead of fatal.
    try:
        import concourse.bass_utils as bu

        _orig_upload = bu.upload_artifacts

        def _safe_upload(tmpdir):
            try:
                return _orig_upload(tmpdir)
            except Exception:
                return tmpdir

        bu.upload_artifacts = _safe_upload
    except Exception:
        pass


_ensure_ntff_hook()


def _equal_chunks(t, maxw=512):
    """Split range(t) into equal-ish chunks of width <= maxw.

    Equal widths keep every chunk wide enough that LDWEIGHTS stays hidden
    under the matmul stream (vs a ragged 512/512/29 split)."""
    n = (t + maxw - 1) // maxw
    base, rem = divmod(t, n)
    out = []
    o = 0
    for i in range(n):
        w = base + (1 if i < rem else 0)
        out.append((o, w))
        o += w
    return out


def _emit_pass(tc, pools, dram, Ttok, pieces, out_eng="gpsimd", head=False):
    """Emit one SwiGLU FFN pass: yT[D,Ttok] = W2^T @ (x@W1 * silu(x@W3))^T.

    dram: dict with xt [128,KD,Ttok] bf16 (p-major packed; for head=True,
          [128,KD,Ttok+2*LEADW] with mi=0's w3/w1 lead columns appended per
          k-chunk), w1/w3 [128,KD,H] bf16, w2 [128,KH,D] bf16,
          y / y_parts [D,*] bf16 DRAM APs.
    head: kernel's first pass — x and the mi=0 weight leads travel in ONE
          host-packed tensor / one DMA (each serial dma_start issue on the
          Sync queue costs ~0.6us at the head, so fewer issues = earlier
          first matmul). mi=0's lhsT then reads from the x tile's tail.
    """
    nc = tc.nc
    chunks = _equal_chunks(Ttok)

    # Resident SBUF tensors (bufs=1 pools; pass 2 reuses the same slots)
    xw = Ttok + 2 * LEADW if head else Ttok
    x_sb = pools["x"].tile([P, KD, xw], mybir.dt.bfloat16, tag="x_sb")
    w1_sb = pools["wA"].tile([P, KD, H], mybir.dt.bfloat16, tag="w1_sb")
    w3_sb = pools["wA"].tile([P, KD, H], mybir.dt.bfloat16, tag="w3_sb")
    w2_sb = pools["wB"].tile([P, KH, D], mybir.dt.bfloat16, tag="w2_sb")
    h2t_sb = pools["h2t"].tile([P, KH, Ttok], mybir.dt.bfloat16, tag="h2t_sb")

    # Input DMAs: large contiguous-per-partition transfers, ordered by first
    # use: the x(+leads) prefix unblocks mi=0, the rest streams behind while
    # the first h-tiles compute. All inputs stay on the Sync HWDGE ring:
    # routing pieces through GpSimd SWDGE was tried and is slower (Q7
    # descriptor generation) and once crashed the device.
    nc.sync.dma_start(out=x_sb[:], in_=dram["xt"])
    a = LEADW if head else 0
    for b in pieces:
        nc.sync.dma_start(out=w3_sb[:, :, a:b], in_=dram["w3"][:, :, a:b])
        nc.sync.dma_start(out=w1_sb[:, :, a:b], in_=dram["w1"][:, :, a:b])
        a = b
    nc.sync.dma_start(out=w2_sb[:], in_=dram["w2"])

    # Phase A: h2T[H, Ttok] = (W1^T x^T) * silu(W3^T x^T), bf16.
    for mi in range(KH):
        hsl = slice(mi * P, (mi + 1) * P)
        for o, nw in chunks:
            # ps3 accumulates FIRST: its sigmoid+mul evict then overlaps ps1's
            # matmuls, leaving only the final h2t mul exposed after ps1 stops.
            ps3 = pools["psA"].tile([P, 512], mybir.dt.float32, tag="ps3", bufs=2)
            ps1 = pools["psA"].tile([P, 512], mybir.dt.float32, tag="ps1", bufs=2)
            # mi=0's stationary weights ride in the x prefix tensor (head)
            lead = head and mi == 0
            for k in range(KD):
                nc.tensor.matmul(
                    ps3[:, :nw],
                    lhsT=x_sb[:, k : k + 1, Ttok : Ttok + LEADW]
                    if lead
                    else w3_sb[:, k : k + 1, hsl],
                    rhs=x_sb[:, k : k + 1, o : o + nw],
                    start=(k == 0),
                    stop=(k == KD - 1),
                )
            for k in range(KD):
                nc.tensor.matmul(
                    ps1[:, :nw],
                    lhsT=x_sb[:, k : k + 1, Ttok + LEADW : Ttok + 2 * LEADW]
                    if lead
                    else w1_sb[:, k : k + 1, hsl],
                    rhs=x_sb[:, k : k + 1, o : o + nw],
                    start=(k == 0),
                    stop=(k == KD - 1),
                )
            # silu = h3 * sigmoid(h3); split sigmoid+mul beats the ACT Silu
            # table on HW (cold-table cost) and matches CoreSim.
            sig = pools["tmp"].tile([P, 512], mybir.dt.float32, tag="sig", bufs=2)
            nc.scalar.activation(
                sig[:, :nw], ps3[:, :nw], mybir.ActivationFunctionType.Sigmoid
            )
            # silu in place: sig <- ps3 * sig (same-engine reuse, fewer tiles)
            nc.vector.tensor_mul(sig[:, :nw], ps3[:, :nw], sig[:, :nw])
            nc.vector.tensor_mul(h2t_sb[:, mi, o : o + nw], ps1[:, :nw], sig[:, :nw])

    # Phase B: yT[D, Ttok] = W2^T @ h2T. Tokens are the moving dim, so the
    # ragged token count costs proportionally; each D-tile's output row block
    # is contiguous in DRAM (one fast output DMA per tile).
    for di in range(D // P):
        dsl = slice(di * P, (di + 1) * P)
        yt = pools["yt"].tile([P, Ttok], mybir.dt.bfloat16, tag="yt", bufs=2)
        for o, nw in chunks:
            ps = pools["psB"].tile([P, 512], mybir.dt.float32, tag="psB", bufs=3)
            for k in range(KH):
                nc.tensor.matmul(
                    ps[:, :nw],
                    lhsT=w2_sb[:, k : k + 1, dsl],
                    rhs=h2t_sb[:, k : k + 1, o : o + nw],
                    start=(k == 0),
                    stop=(k == KH - 1),
                )
            nc.vector.tensor_copy(out=yt[:, o : o + nw], in_=ps[:, :nw])
            # Per-chunk output tensors (when provided) are each contiguous
            # in DRAM, so chunk outputs ship as soon as they're evicted and
            # only a half-size transfer (+HBM write receipt) trails the last
            # matmul. (A column slice of one [D,C] tensor would be a strided
            # write — the measured slow DMA pattern.)
            if "y_parts" in dram:
                ci = next(i for i, (po, _) in enumerate(chunks) if po == o)
                getattr(nc, out_eng).dma_start(
                    out=dram["y_parts"][ci][dsl, :], in_=yt[:, o : o + nw]
                )
        # Output engine choice (measured): the Scalar HWDGE ring drains
        # nearly serially (~25-30 GB/s) so it's avoided. Pass 1 outputs ride
        # the GpSimd SWDGE ring (idle, parallel; its slow-engine-7/15
        # straggler is harmless mid-kernel, and it can't delay the weight
        # stream on Sync). Pass 2 outputs ride the Sync HWDGE ring, which is
        # empty once the input stream is done and drains in parallel with no
        # SWDGE straggler — keeping the final output off the critical tail.
        if "y_parts" not in dram:
            getattr(nc, out_eng).dma_start(out=dram["y"][dsl, :], in_=yt[:])


def _build_nc(C, SS):
    """Build the per-core Bass program: shared FFN ([SS] tokens) + routed FFN ([C])."""
    nc = bacc.Bacc("TRN2", target_bir_lowering=False, debug=False)

    bf = mybir.dt.bfloat16
    shared = {
        "xt": nc.dram_tensor(
            "xst", [P, KD, SS + 2 * LEADW], bf, kind="ExternalInput"
        ).ap(),
        "w1": nc.dram_tensor("ws1", [P, KD, H], bf, kind="ExternalInput").ap(),
        "w3": nc.dram_tensor("ws3", [P, KD, H], bf, kind="ExternalInput").ap(),
        "w2": nc.dram_tensor("ws2", [P, KH, D], bf, kind="ExternalInput").ap(),
        "y": nc.dram_tensor("yst", [D, SS], bf, kind="ExternalOutput").ap(),
    }
    routed = {
        "xt": nc.dram_tensor("xgt", [P, KD, C], bf, kind="ExternalInput").ap(),
        "w1": nc.dram_tensor("w1", [P, KD, H], bf, kind="ExternalInput").ap(),
        "w3": nc.dram_tensor("w3", [P, KD, H], bf, kind="ExternalInput").ap(),
        "w2": nc.dram_tensor("w2", [P, KH, D], bf, kind="ExternalInput").ap(),
        "y_parts": [
            nc.dram_tensor(f"ygt{i}", [D, nw], bf, kind="ExternalOutput").ap()
            for i, (_, nw) in enumerate(_equal_chunks(C))
        ],
    }

    with tile.TileContext(nc) as tc, ExitStack() as ctx:
        pools = {
            "x": ctx.enter_context(tc.tile_pool(name="x", bufs=1)),
            "xg": ctx.enter_context(tc.tile_pool(name="xg", bufs=1)),
            "wA": ctx.enter_context(tc.tile_pool(name="wA", bufs=1)),
            "wB": ctx.enter_context(tc.tile_pool(name="wB", bufs=1)),
            "h2t": ctx.enter_context(tc.tile_pool(name="h2t", bufs=1)),
            "tmp": ctx.enter_context(tc.tile_pool(name="tmp", bufs=2)),
            "yt": ctx.enter_context(tc.tile_pool(name="yt", bufs=2)),
            "psA": ctx.enter_context(tc.tile_pool(name="psA", bufs=2, space="PSUM")),
            "psB": ctx.enter_context(tc.tile_pool(name="psB", bufs=3, space="PSUM")),
        }
        # HAM warm-up: cold matmuls on a zeroed tile while the input DMAs
        # stream in, so the PE clock-gate is at 8/8 when real work starts.
        warm = pools["tmp"].tile([P, P], mybir.dt.bfloat16, tag="warm", bufs=1)
        nc.vector.memset(warm[:], 0.0)
        wps = pools["psB"].tile([P, P], mybir.dt.float32, tag="psB", name="wps")
        for _ in range(WARMUP_MM):
            nc.tensor.matmul(wps[:], lhsT=warm[:], rhs=warm[:], start=True, stop=True)
        # Shared pass first: its 1MB x block + weight leads unblock the PE
        # soonest. The routed pass's x prefetches early into its own slot
        # ("xg" pool); its weights stream into the freed "wA"/"wB" slots
        # during the shared pass's phase B.
        sh_pools = dict(pools)
        _emit_pass(tc, sh_pools, shared, SS, PIECES_FINE, head=True)
        ro_pools = dict(pools)
        ro_pools["x"] = pools["xg"]
        _emit_pass(tc, ro_pools, routed, C, PIECES_COARSE, out_eng="sync")

    nc.compile()
    return nc


def _route(x, Wr, rb):
    """Replicate the reference router. Returns (idx [T,2] int, w [T,2] f32).

    Uses jax on CPU with the exact expressions from the reference so the top-2
    selection bit-matches a CPU-run reference (min 2nd-vs-3rd logit gap in this
    problem is ~1e-6, so the selection must match the reference's fp32 math).
    Falls back to numpy float64 if jax-cpu is unavailable.
    """
    try:
        import jax
        import jax.numpy as jnp

        cpu = jax.devices("cpu")[0]
        with jax.default_device(cpu):
            xl = jnp.asarray(np.asarray(x))
            wr = jnp.asarray(np.asarray(Wr))
            rbj = jnp.asarray(np.asarray(rb))
            logits = jnp.einsum("bsd,de->bse", xl, wr) * SCALE
            _, idx = jax.lax.top_k(logits + rbj, TOPK)
            gathered = jnp.take_along_axis(logits, idx, axis=-1)
            w = jax.nn.softmax(gathered, axis=-1)
        idx = np.asarray(idx).reshape(-1, TOPK)
        w = np.asarray(w, dtype=np.float32).reshape(-1, TOPK)
        return idx, w
    except Exception:
        xf = np.asarray(x, np.float64).reshape(-1, D)
        logits = (xf @ np.asarray(Wr, np.float64)) * SCALE
        biased = logits + np.asarray(rb, np.float64)
        idx = np.argsort(-biased, axis=-1)[:, :TOPK]
        g = np.take_along_axis(logits, idx, axis=-1)
        g = g - g.max(axis=-1, keepdims=True)
        wexp = np.exp(g)
        w = (wexp / wexp.sum(axis=-1, keepdims=True)).astype(np.float32)
        return idx, w


def _pack_p(a2d, kchunks):
    """[R, cols] -> p-major [128, kchunks, cols] (R = kchunks*128)."""
    cols = a2d.shape[1]
    return np.ascontiguousarray(
        a2d.reshape(kchunks, P, cols).transpose(1, 0, 2)
    )


def kernel(x, Wr, rb, W1, W2, W3, Ws1, Ws2, Ws3):
    global LAST_RESULTS
    x = np.asarray(x, np.float32)
    Wr = np.asarray(Wr, np.float32)
    rb = np.asarray(rb, np.float32)
    W1 = np.asarray(W1, np.float32)
    W2 = np.asarray(W2, np.float32)
    W3 = np.asarray(W3, np.float32)
    Ws1 = np.asarray(Ws1, np.float32)
    Ws2 = np.asarray(Ws2, np.float32)
    Ws3 = np.asarray(Ws3, np.float32)

    T = B * S
    xf = x.reshape(T, D)

    # ---- Router (host, exact) ----
    idx, w = _route(x, Wr, rb)

    # ---- Shard ----
    toks = [np.nonzero((idx == e).any(axis=1))[0] for e in range(E)]
    wtok = [
        w[toks[e], :][idx[toks[e], :] == e].astype(np.float32) for e in range(E)
    ]
    counts = [len(t) for t in toks]
    # Cap each expert's device block at the balanced mean (T*K/E = 1024); the
    # few overflow tokens of over-subscribed experts (~1.5% of routed work)
    # are computed on the host in fp32. Every core then does exactly the mean
    # routed load instead of all cores padding to the max expert's count, and
    # the token chunks split into even 512s.
    CAP = T * TOPK // E
    C = max(256, min(max(counts), CAP))
    dev_counts = [min(c, C) for c in counts]
    SS = T // NCORES

    xf_bf = xf.astype(BF16)
    ws1p = _pack_p(Ws1.astype(BF16), KD)
    ws3p = _pack_p(Ws3.astype(BF16), KD)
    ws2p = _pack_p(Ws2.astype(BF16), KH)
    wlead = np.concatenate([ws3p[:, :, :LEADW], ws1p[:, :, :LEADW]], axis=2)
    in_maps = []
    for e in range(E):
        xg = np.zeros((C, D), dtype=BF16)
        xg[: dev_counts[e]] = xf_bf[toks[e][: dev_counts[e]]]
        in_maps.append(
            {
                "xst": np.concatenate(
                    [
                        _pack_p(
                            np.ascontiguousarray(xf_bf[e * SS : (e + 1) * SS].T),
                            KD,
                        ),
                        wlead,
                    ],
                    axis=2,
                ),
                "ws1": ws1p,
                "ws3": ws3p,
                "ws2": ws2p,
                "xgt": _pack_p(np.ascontiguousarray(xg.T), KD),
                "w1": _pack_p(W1[e].astype(BF16), KD),
                "w3": _pack_p(W3[e].astype(BF16), KD),
                "w2": _pack_p(W2[e].astype(BF16), KH),
            }
        )

    # ---- Device ----
    key = (C, SS)
    if key not in _NC_CACHE:
        _NC_CACHE[key] = _build_nc(C, SS)
    nc = _NC_CACHE[key]
    res = run_bass_kernel_spmd(nc, in_maps, list(range(NCORES)))
    LAST_RESULTS = res

    # ---- Combine (host) ----
    out = np.empty((T, D), dtype=np.float32)
    for e in range(E):
        out[e * SS : (e + 1) * SS] = res.results[e]["yst"].astype(np.float32).T
    nyg = len(_equal_chunks(C))
    for e in range(E):
        yg = np.concatenate(
            [res.results[e][f"ygt{i}"].astype(np.float32).T for i in range(nyg)],
            axis=0,
        )[: dev_counts[e]]
        out[toks[e][: dev_counts[e]]] += wtok[e][: dev_counts[e], None] * yg
        if dev_counts[e] < counts[e]:
            # fp32 host FFN for the capacity-overflow tokens of this expert
            ot = toks[e][dev_counts[e] :]
            xe = xf[ot]
            h3 = xe @ W3[e]
            h = (xe @ W1[e]) * (h3 / (1.0 + np.exp(-h3)))
            out[ot] += wtok[e][dev_counts[e] :, None] * (h @ W2[e])
    return out.reshape(B, S, D)


# revision 59
# speedup vs baseline: 1.0059x; 1.0059x over previous
"""MoE layer (routed top-2 experts + shared SwiGLU expert) on 8 TRN2 NeuronCores.

Sharding strategy (per spec hint):
  - Routed experts: expert-parallel. Core e holds W1/W2/W3[e]; the host computes
    the router (bit-matching the reference's jax fp32 computation on CPU), gathers
    each expert's assigned tokens (top-2 of 8 per token => ~T/4 tokens per expert),
    and ships a [C, D] token block per core (C = max expert count). This is exact
    vs. the dense reference since w_full is zero for non-selected experts.
  - Shared expert: data-parallel on tokens. Core e processes tokens
    [e*T/8, (e+1)*T/8) through the full shared SwiGLU (weights replicated).
  - Combine: host scatter-add of weighted routed outputs + shared outputs.

Device kernel per core: two SwiGLU FFN passes (shared block, routed block):
    hT = (W1^T x^T) [H, Ttok]  (PSUM f32, accumulated over D/128 chunks)
    h2T = hT * silu(h3T)       (ACT sigmoid + DVE muls, cast to bf16)
    yT = W2^T @ h2T            [D, Ttok]  (token dim moving => ragged counts
                               cost proportionally; output rows contiguous)
All matmuls in bf16 with fp32 PSUM accumulation; outputs written bf16.

All inputs are host-packed p-major ([128, k, cols] matching the SBUF tile
layout) so each tensor loads with one large contiguous-per-partition DMA:
the lead-in to the first real matmul is ~4us instead of ~10us of small
descriptor-limited transfers.
"""

from contextlib import ExitStack

import numpy as np
import ml_dtypes

import concourse.bacc as bacc
import concourse.tile as tile
from concourse import mybir
from concourse.bass_utils import run_bass_kernel_spmd

# Problem constants (hardcoded per the self-contained-kernel contract)
B, S, D, H, E, TOPK = 2, 2048, 1024, 2048, 8, 2
SCALE = 1.0 / float(np.sqrt(D))
NCORES = 8
P = 128
KD = D // P     # contraction chunks for phase A (8)
KH = H // P     # h tiles / phase-B contraction chunks (16)
BF16 = ml_dtypes.bfloat16

# test.py introspection: last BassKernelResults (exec_time_ns when BASS_TRACE=1)
LAST_RESULTS = None

_NC_CACHE = {}

WARMUP_MM = 44    # ~4.5us of N=128 matmuls: trips the HAM clock-gate to 8/8
                  # and keeps the PE busy until the first input DMAs land
                  # (an idle gap >3.4us would re-throttle the clock).
LEADW = 128       # pass-1 lead piece: shipped as its own host-packed
                  # contiguous tensor (2KB/partition lines), so it can be one
                  # h-tile wide without tripping the 512B descriptor floor —
                  # the first matmul then needs only x + 2*0.25MB of weights.
# Remaining weight-column DMA piece boundaries for pass 1: sized so later
# h-tiles stay ahead of the PE's ~3.4us/tile consumption; every slice
# segment is >=768B. Pass 2 is WAR-gated on pass 1 anyway so it uses coarse
# pieces (fewer issues).
PIECES_FINE = (512, 1280, H)
PIECES_COARSE = (512, H)


def _ensure_ntff_hook():
    """run_bass_kernel_spmd(trace=True) imports antenv.axon_hooks, which this
    image's antenv lacks. Install a stub (wired to the ctypes NTFF profiler if
    available) so a BASS_TRACE=1 environment doesn't crash the kernel."""
    import sys
    import types

    try:
        import antenv.axon_hooks  # noqa: F401

        return
    except ImportError:
        pass
    try:
        import antenv
    except ImportError:
        return
    mod = types.ModuleType("antenv.axon_hooks")
    holder = [None]
    mod.set_axon_ntff_profile_hook = lambda h: holder.__setitem__(0, h)
    mod.get_axon_ntff_profile_hook = lambda: holder[0]
    sys.modules["antenv.axon_hooks"] = mod
    antenv.axon_hooks = mod
    try:
        import trn_agent_boot.trn_boot as tb

        mod.set_axon_ntff_profile_hook(
            tb._ntff_profile_via_ctypes("/opt/axon/libaxon_pjrt.so")
        )
    except Exception:
        pass
    # In hook-less images the artifact share upload is likely unavailable too;
    # make the trace path's upload best-effort instead of fatal.
    try:
        import concourse.bass_utils as bu

        _orig_upload = bu.upload_artifacts

        def _safe_upload(tmpdir):
            try:
                return _orig_upload(tmpdir)
            except Exception:
                return tmpdir

        bu.upload_artifacts = _safe_upload
    except Exception:
        pass


_ensure_ntff_hook()


def _equal_chunks(t, maxw=512):
    """Split range(t) into equal-ish chunks of width <= maxw.

    Equal widths keep every chunk wide enough that LDWEIGHTS stays hidden
    under the matmul stream (vs a ragged 512/512/29 split)."""
    n = (t + maxw - 1) // maxw
    base, rem = divmod(t, n)
    out = []
    o = 0
    for i in range(n):
        w = base + (1 if i < rem else 0)
        out.append((o, w))
        o += w
    return out


def _emit_pass(tc, pools, dram, Ttok, pieces, out_eng="gpsimd", head=False):
    """Emit one SwiGLU FFN pass: yT[D,Ttok] = W2^T @ (x@W1 * silu(x@W3))^T.

    dram: dict with xt [128,KD,Ttok] bf16 (p-major packed), w1/w3 [128,KD,H]
          bf16, w2 [128,KH,D] bf16, y / y_parts [D,*] bf16 DRAM APs.
    head: kernel's first pass — the mi=0 weight leads arrive via dedicated
          host-packed contiguous tensors (w3l/w1l), minimizing the
          first-matmul DMA prefix. (Merging x+leads into one tensor/DMA was
          tried but could not be validated faster under the downclock
          lottery, so the twice-confirmed split form is kept.)
    """
    nc = tc.nc
    chunks = _equal_chunks(Ttok)

    # Resident SBUF tensors (bufs=1 pools; pass 2 reuses the same slots)
    x_sb = pools["x"].tile([P, KD, Ttok], mybir.dt.bfloat16, tag="x_sb")
    w1_sb = pools["wA"].tile([P, KD, H], mybir.dt.bfloat16, tag="w1_sb")
    w3_sb = pools["wA"].tile([P, KD, H], mybir.dt.bfloat16, tag="w3_sb")
    w2_sb = pools["wB"].tile([P, KH, D], mybir.dt.bfloat16, tag="w2_sb")
    h2t_sb = pools["h2t"].tile([P, KH, Ttok], mybir.dt.bfloat16, tag="h2t_sb")

    # Input DMAs: large contiguous-per-partition transfers, ordered by first
    # use: the x(+leads) prefix unblocks mi=0, the rest streams behind while
    # the first h-tiles compute. All inputs stay on the Sync HWDGE ring:
    # routing pieces through GpSimd SWDGE was tried and is slower (Q7
    # descriptor generation) and once crashed the device.
    nc.sync.dma_start(out=x_sb[:], in_=dram["xt"])
    a = 0
    if head:
        # Dedicated packed lead tensors: contiguous per partition, so the
        # lead can be a single h-tile (0.25MB) — smallest possible prefix.
        nc.sync.dma_start(out=w3_sb[:, :, :LEADW], in_=dram["w3l"])
        nc.sync.dma_start(out=w1_sb[:, :, :LEADW], in_=dram["w1l"])
        a = LEADW
    for b in pieces:
        nc.sync.dma_start(out=w3_sb[:, :, a:b], in_=dram["w3"][:, :, a:b])
        nc.sync.dma_start(out=w1_sb[:, :, a:b], in_=dram["w1"][:, :, a:b])
        a = b
    nc.sync.dma_start(out=w2_sb[:], in_=dram["w2"])

    # Phase A: h2T[H, Ttok] = (W1^T x^T) * silu(W3^T x^T), bf16.
    for mi in range(KH):
        hsl = slice(mi * P, (mi + 1) * P)
        for o, nw in chunks:
            # ps3 accumulates FIRST: its sigmoid+mul evict then overlaps ps1's
            # matmuls, leaving only the final h2t mul exposed after ps1 stops.
            ps3 = pools["psA"].tile([P, 512], mybir.dt.float32, tag="ps3", bufs=2)
            ps1 = pools["psA"].tile([P, 512], mybir.dt.float32, tag="ps1", bufs=2)
            for k in range(KD):
                nc.tensor.matmul(
                    ps3[:, :nw],
                    lhsT=w3_sb[:, k : k + 1, hsl],
                    rhs=x_sb[:, k : k + 1, o : o + nw],
                    start=(k == 0),
                    stop=(k == KD - 1),
                )
            for k in range(KD):
                nc.tensor.matmul(
                    ps1[:, :nw],
                    lhsT=w1_sb[:, k : k + 1, hsl],
                    rhs=x_sb[:, k : k + 1, o : o + nw],
                    start=(k == 0),
                    stop=(k == KD - 1),
                )
            # silu = h3 * sigmoid(h3); split sigmoid+mul beats the ACT Silu
            # table on HW (cold-table cost) and matches CoreSim.
            sig = pools["tmp"].tile([P, 512], mybir.dt.float32, tag="sig", bufs=2)
            nc.scalar.activation(
                sig[:, :nw], ps3[:, :nw], mybir.ActivationFunctionType.Sigmoid
            )
            # silu in place: sig <- ps3 * sig (same-engine reuse, fewer tiles)
            nc.vector.tensor_mul(sig[:, :nw], ps3[:, :nw], sig[:, :nw])
            nc.vector.tensor_mul(h2t_sb[:, mi, o : o + nw], ps1[:, :nw], sig[:, :nw])

    # Phase B: yT[D, Ttok] = W2^T @ h2T. Tokens are the moving dim, so the
    # ragged token count costs proportionally; each D-tile's output row block
    # is contiguous in DRAM (one fast output DMA per tile).
    for di in range(D // P):
        dsl = slice(di * P, (di + 1) * P)
        yt = pools["yt"].tile([P, Ttok], mybir.dt.bfloat16, tag="yt", bufs=2)
        for o, nw in chunks:
            ps = pools["psB"].tile([P, 512], mybir.dt.float32, tag="psB", bufs=3)
            for k in range(KH):
                nc.tensor.matmul(
                    ps[:, :nw],
                    lhsT=w2_sb[:, k : k + 1, dsl],
                    rhs=h2t_sb[:, k : k + 1, o : o + nw],
                    start=(k == 0),
                    stop=(k == KH - 1),
                )
            nc.vector.tensor_copy(out=yt[:, o : o + nw], in_=ps[:, :nw])
            # Per-chunk output tensors (when provided) are each contiguous
            # in DRAM, so chunk outputs ship as soon as they're evicted and
            # only a half-size transfer (+HBM write receipt) trails the last
            # matmul. (A column slice of one [D,C] tensor would be a strided
            # write — the measured slow DMA pattern.)
            if "y_parts" in dram:
                ci = next(i for i, (po, _) in enumerate(chunks) if po == o)
                getattr(nc, out_eng).dma_start(
                    out=dram["y_parts"][ci][dsl, :], in_=yt[:, o : o + nw]
                )
        # Output engine choice (measured): the Scalar HWDGE ring drains
        # nearly serially (~25-30 GB/s) so it's avoided. Pass 1 outputs ride
        # the GpSimd SWDGE ring (idle, parallel; its slow-engine-7/15
        # straggler is harmless mid-kernel, and it can't delay the weight
        # stream on Sync). Pass 2 outputs ride the Sync HWDGE ring, which is
        # empty once the input stream is done and drains in parallel with no
        # SWDGE straggler — keeping the final output off the critical tail.
        if "y_parts" not in dram:
            getattr(nc, out_eng).dma_start(out=dram["y"][dsl, :], in_=yt[:])


def _build_nc(C, SS):
    """Build the per-core Bass program: shared FFN ([SS] tokens) + routed FFN ([C])."""
    nc = bacc.Bacc("TRN2", target_bir_lowering=False, debug=False)

    bf = mybir.dt.bfloat16
    shared = {
        "xt": nc.dram_tensor("xst", [P, KD, SS], bf, kind="ExternalInput").ap(),
        "w1": nc.dram_tensor("ws1", [P, KD, H], bf, kind="ExternalInput").ap(),
        "w3": nc.dram_tensor("ws3", [P, KD, H], bf, kind="ExternalInput").ap(),
        "w1l": nc.dram_tensor("ws1l", [P, KD, LEADW], bf, kind="ExternalInput").ap(),
        "w3l": nc.dram_tensor("ws3l", [P, KD, LEADW], bf, kind="ExternalInput").ap(),
        "w2": nc.dram_tensor("ws2", [P, KH, D], bf, kind="ExternalInput").ap(),
        "y": nc.dram_tensor("yst", [D, SS], bf, kind="ExternalOutput").ap(),
    }
    routed = {
        "xt": nc.dram_tensor("xgt", [P, KD, C], bf, kind="ExternalInput").ap(),
        "w1": nc.dram_tensor("w1", [P, KD, H], bf, kind="ExternalInput").ap(),
        "w3": nc.dram_tensor("w3", [P, KD, H], bf, kind="ExternalInput").ap(),
        "w2": nc.dram_tensor("w2", [P, KH, D], bf, kind="ExternalInput").ap(),
        "y_parts": [
            nc.dram_tensor(f"ygt{i}", [D, nw], bf, kind="ExternalOutput").ap()
            for i, (_, nw) in enumerate(_equal_chunks(C))
        ],
    }

    with tile.TileContext(nc) as tc, ExitStack() as ctx:
        pools = {
            "x": ctx.enter_context(tc.tile_pool(name="x", bufs=1)),
            "xg": ctx.enter_context(tc.tile_pool(name="xg", bufs=1)),
            "wA": ctx.enter_context(tc.tile_pool(name="wA", bufs=1)),
            "wB": ctx.enter_context(tc.tile_pool(name="wB", bufs=1)),
            "h2t": ctx.enter_context(tc.tile_pool(name="h2t", bufs=1)),
            "tmp": ctx.enter_context(tc.tile_pool(name="tmp", bufs=2)),
            "yt": ctx.enter_context(tc.tile_pool(name="yt", bufs=2)),
            "psA": ctx.enter_context(tc.tile_pool(name="psA", bufs=2, space="PSUM")),
            "psB": ctx.enter_context(tc.tile_pool(name="psB", bufs=3, space="PSUM")),
        }
        # HAM warm-up: cold matmuls on a zeroed tile while the input DMAs
        # stream in, so the PE clock-gate is at 8/8 when real work starts.
        warm = pools["tmp"].tile([P, P], mybir.dt.bfloat16, tag="warm", bufs=1)
        nc.vector.memset(warm[:], 0.0)
        wps = pools["psB"].tile([P, P], mybir.dt.float32, tag="psB", name="wps")
        for _ in range(WARMUP_MM):
            nc.tensor.matmul(wps[:], lhsT=warm[:], rhs=warm[:], start=True, stop=True)
        # Shared pass first: its 1MB x block + weight leads unblock the PE
        # soonest. The routed pass's x prefetches early into its own slot
        # ("xg" pool); its weights stream into the freed "wA"/"wB" slots
        # during the shared pass's phase B.
        sh_pools = dict(pools)
        _emit_pass(tc, sh_pools, shared, SS, PIECES_FINE, head=True)
        ro_pools = dict(pools)
        ro_pools["x"] = pools["xg"]
        _emit_pass(tc, ro_pools, routed, C, PIECES_COARSE, out_eng="sync")

    nc.compile()
    return nc


def _route(x, Wr, rb):
    """Replicate the reference router. Returns (idx [T,2] int, w [T,2] f32).

    Uses jax on CPU with the exact expressions from the reference so the top-2
    selection bit-matches a CPU-run reference (min 2nd-vs-3rd logit gap in this
    problem is ~1e-6, so the selection must match the reference's fp32 math).
    Falls back to numpy float64 if jax-cpu is unavailable.
    """
    try:
        import jax
        import jax.numpy as jnp

        cpu = jax.devices("cpu")[0]
        with jax.default_device(cpu):
            xl = jnp.asarray(np.asarray(x))
            wr = jnp.asarray(np.asarray(Wr))
            rbj = jnp.asarray(np.asarray(rb))
            logits = jnp.einsum("bsd,de->bse", xl, wr) * SCALE
            _, idx = jax.lax.top_k(logits + rbj, TOPK)
            gathered = jnp.take_along_axis(logits, idx, axis=-1)
            w = jax.nn.softmax(gathered, axis=-1)
        idx = np.asarray(idx).reshape(-1, TOPK)
        w = np.asarray(w, dtype=np.float32).reshape(-1, TOPK)
        return idx, w
    except Exception:
        xf = np.asarray(x, np.float64).reshape(-1, D)
        logits = (xf @ np.asarray(Wr, np.float64)) * SCALE
        biased = logits + np.asarray(rb, np.float64)
        idx = np.argsort(-biased, axis=-1)[:, :TOPK]
        g = np.take_along_axis(logits, idx, axis=-1)
        g = g - g.max(axis=-1, keepdims=True)
        wexp = np.exp(g)
        w = (wexp / wexp.sum(axis=-1, keepdims=True)).astype(np.float32)
        return idx, w


def _pack_p(a2d, kchunks):
    """[R, cols] -> p-major [128, kchunks, cols] (R = kchunks*128)."""
    cols = a2d.shape[1]
    return np.ascontiguousarray(
        a2d.reshape(kchunks, P, cols).transpose(1, 0, 2)
    )


def kernel(x, Wr, rb, W1, W2, W3, Ws1, Ws2, Ws3):
    global LAST_RESULTS
    x = np.asarray(x, np.float32)
    Wr = np.asarray(Wr, np.float32)
    rb = np.asarray(rb, np.float32)
    W1 = np.asarray(W1, np.float32)
    W2 = np.asarray(W2, np.float32)
    W3 = np.asarray(W3, np.float32)
    Ws1 = np.asarray(Ws1, np.float32)
    Ws2 = np.asarray(Ws2, np.float32)
    Ws3 = np.asarray(Ws3, np.float32)

    T = B * S
    xf = x.reshape(T, D)

    # ---- Router (host, exact) ----
    idx, w = _route(x, Wr, rb)

    # ---- Shard ----
    toks = [np.nonzero((idx == e).any(axis=1))[0] for e in range(E)]
    wtok = [
        w[toks[e], :][idx[toks[e], :] == e].astype(np.float32) for e in range(E)
    ]
    counts = [len(t) for t in toks]
    # Cap each expert's device block at the balanced mean (T*K/E = 1024); the
    # few overflow tokens of over-subscribed experts (~1.5% of routed work)
    # are computed on the host in fp32. Every core then does exactly the mean
    # routed load instead of all cores padding to the max expert's count, and
    # the token chunks split into even 512s.
    CAP = T * TOPK // E
    C = max(256, min(max(counts), CAP))
    dev_counts = [min(c, C) for c in counts]
    SS = T // NCORES

    xf_bf = xf.astype(BF16)
    ws1p = _pack_p(Ws1.astype(BF16), KD)
    ws3p = _pack_p(Ws3.astype(BF16), KD)
    ws2p = _pack_p(Ws2.astype(BF16), KH)
    ws1l = np.ascontiguousarray(ws1p[:, :, :LEADW])
    ws3l = np.ascontiguousarray(ws3p[:, :, :LEADW])
    in_maps = []
    for e in range(E):
        xg = np.zeros((C, D), dtype=BF16)
        xg[: dev_counts[e]] = xf_bf[toks[e][: dev_counts[e]]]
        in_maps.append(
            {
                "xst": _pack_p(
                    np.ascontiguousarray(xf_bf[e * SS : (e + 1) * SS].T), KD
                ),
                "ws1": ws1p,
                "ws3": ws3p,
                "ws1l": ws1l,
                "ws3l": ws3l,
                "ws2": ws2p,
                "xgt": _pack_p(np.ascontiguousarray(xg.T), KD),
                "w1": _pack_p(W1[e].astype(BF16), KD),
                "w3": _pack_p(W3[e].astype(BF16), KD),
                "w2": _pack_p(W2[e].astype(BF16), KH),
            }
        )

    # ---- Device ----
    key = (C, SS)
    if key not in _NC_CACHE:
        _NC_CACHE[key] = _build_nc(C, SS)
    nc = _NC_CACHE[key]
    res = run_bass_kernel_spmd(nc, in_maps, list(range(NCORES)))
    LAST_RESULTS = res

    # ---- Combine (host) ----
    out = np.empty((T, D), dtype=np.float32)
    for e in range(E):
        out[e * SS : (e + 1) * SS] = res.results[e]["yst"].astype(np.float32).T
    nyg = len(_equal_chunks(C))
    for e in range(E):
        yg = np.concatenate(
            [res.results[e][f"ygt{i}"].astype(np.float32).T for i in range(nyg)],
            axis=0,
        )[: dev_counts[e]]
        out[toks[e][: dev_counts[e]]] += wtok[e][: dev_counts[e], None] * yg
        if dev_counts[e] < counts[e]:
            # fp32 host FFN for the capacity-overflow tokens of this expert
            ot = toks[e][dev_counts[e] :]
            xe = xf[ot]
            h3 = xe @ W3[e]
            h = (xe @ W1[e]) * (h3 / (1.0 + np.exp(-h3)))
            out[ot] += wtok[e][dev_counts[e] :, None] * (h @ W2[e])
    return out.reshape(B, S, D)


# revision 62
# speedup vs baseline: 1.2002x; 1.1932x over previous
"""MoE layer (routed top-2 experts + shared SwiGLU expert) on 8 TRN2 NeuronCores.

Sharding strategy (per spec hint):
  - Routed experts: expert-parallel. Core e holds W1/W2/W3[e]; the host computes
    the router (bit-matching the reference's jax fp32 computation on CPU), gathers
    each expert's assigned tokens (top-2 of 8 per token => ~T/4 tokens per expert),
    and ships a [C, D] token block per core (C = max expert count). This is exact
    vs. the dense reference since w_full is zero for non-selected experts.
  - Shared expert: data-parallel on tokens. Core e processes tokens
    [e*T/8, (e+1)*T/8) through the full shared SwiGLU (weights replicated).
  - Combine: host scatter-add of weighted routed outputs + shared outputs.

Device kernel per core: two SwiGLU FFN passes (shared block, routed block):
    hT = (W1^T x^T) [H, Ttok]  (PSUM f32, accumulated over D/128 chunks)
    h2T = hT * silu(h3T)       (ACT sigmoid + DVE muls, cast to bf16)
    yT = W2^T @ h2T            [D, Ttok]  (token dim moving => ragged counts
                               cost proportionally; output rows contiguous)
All matmuls in bf16 with fp32 PSUM accumulation; outputs written bf16.

All inputs are host-packed p-major ([128, k, cols] matching the SBUF tile
layout) so each tensor loads with one large contiguous-per-partition DMA:
the lead-in to the first real matmul is ~4us instead of ~10us of small
descriptor-limited transfers.
"""

from contextlib import ExitStack

import numpy as np
import ml_dtypes

import concourse.bacc as bacc
import concourse.tile as tile
from concourse import mybir
from concourse.bass_utils import run_bass_kernel_spmd

# Problem constants (hardcoded per the self-contained-kernel contract)
B, S, D, H, E, TOPK = 2, 2048, 1024, 2048, 8, 2
SCALE = 1.0 / float(np.sqrt(D))
NCORES = 8
P = 128
KD = D // P     # contraction chunks for phase A (8)
KH = H // P     # h tiles / phase-B contraction chunks (16)
BF16 = ml_dtypes.bfloat16

# test.py introspection: last BassKernelResults (exec_time_ns when BASS_TRACE=1)
LAST_RESULTS = None

_NC_CACHE = {}

WARMUP_MM = 44    # ~4.5us of N=128 matmuls: trips the HAM clock-gate to 8/8
                  # and keeps the PE busy until the first input DMAs land
                  # (an idle gap >3.4us would re-throttle the clock).
LEADW = 128       # pass-1 lead piece: shipped as its own host-packed
                  # contiguous tensor (2KB/partition lines), so it can be one
                  # h-tile wide without tripping the 512B descriptor floor —
                  # the first matmul then needs only x + 2*0.25MB of weights.
# Remaining weight-column DMA piece boundaries for pass 1: sized so later
# h-tiles stay ahead of the PE's ~3.4us/tile consumption; every slice
# segment is >=768B. Pass 2 is WAR-gated on pass 1 anyway so it uses coarse
# pieces (fewer issues).
PIECES_FINE = (512, 1280, H)
PIECES_COARSE = (512, H)


def _ensure_ntff_hook():
    """run_bass_kernel_spmd(trace=True) imports antenv.axon_hooks, which this
    image's antenv lacks. Install a stub (wired to the ctypes NTFF profiler if
    available) so a BASS_TRACE=1 environment doesn't crash the kernel."""
    import sys
    import types

    try:
        import antenv.axon_hooks  # noqa: F401

        return
    except ImportError:
        pass
    try:
        import antenv
    except ImportError:
        return
    mod = types.ModuleType("antenv.axon_hooks")
    holder = [None]
    mod.set_axon_ntff_profile_hook = lambda h: holder.__setitem__(0, h)
    mod.get_axon_ntff_profile_hook = lambda: holder[0]
    sys.modules["antenv.axon_hooks"] = mod
    antenv.axon_hooks = mod
    try:
        import trn_agent_boot.trn_boot as tb

        mod.set_axon_ntff_profile_hook(
            tb._ntff_profile_via_ctypes("/opt/axon/libaxon_pjrt.so")
        )
    except Exception:
        pass
    # In hook-less images the artifact share upload is likely unavailable too;
    # make the trace path's upload best-effort instead of fatal.
    try:
        import concourse.bass_utils as bu

        _orig_upload = bu.upload_artifacts

        def _safe_upload(tmpdir):
            try:
                return _orig_upload(tmpdir)
            except Exception:
                return tmpdir

        bu.upload_artifacts = _safe_upload
    except Exception:
        pass


_ensure_ntff_hook()


def _equal_chunks(t, maxw=512):
    """Split range(t) into equal-ish chunks of width <= maxw.

    Equal widths keep every chunk wide enough that LDWEIGHTS stays hidden
    under the matmul stream (vs a ragged 512/512/29 split)."""
    n = (t + maxw - 1) // maxw
    base, rem = divmod(t, n)
    out = []
    o = 0
    for i in range(n):
        w = base + (1 if i < rem else 0)
        out.append((o, w))
        o += w
    return out


def _emit_pass(tc, pools, dram, Ttok, pieces, out_eng="scalar", head=False):
    """Emit one SwiGLU FFN pass: yT[D,Ttok] = W2^T @ (x@W1 * silu(x@W3))^T.

    dram: dict with xt [128,KD,Ttok] bf16 (p-major packed), w1/w3 [128,KD,H]
          bf16, w2 [128,KH,D] bf16, y / y_parts [D,*] bf16 DRAM APs.
    head: kernel's first pass — the mi=0 weight leads arrive via dedicated
          host-packed contiguous tensors (w3l/w1l), minimizing the
          first-matmul DMA prefix. (Merging x+leads into one tensor/DMA was
          tried but could not be validated faster under the downclock
          lottery, so the twice-confirmed split form is kept.)
    """
    nc = tc.nc
    chunks = _equal_chunks(Ttok)

    # Resident SBUF tensors (bufs=1 pools; pass 2 reuses the same slots)
    x_sb = pools["x"].tile([P, KD, Ttok], mybir.dt.bfloat16, tag="x_sb")
    w1_sb = pools["wA"].tile([P, KD, H], mybir.dt.bfloat16, tag="w1_sb")
    w3_sb = pools["wA"].tile([P, KD, H], mybir.dt.bfloat16, tag="w3_sb")
    w2_sb = pools["wB"].tile([P, KH, D], mybir.dt.bfloat16, tag="w2_sb")
    h2t_sb = pools["h2t"].tile([P, KH, Ttok], mybir.dt.bfloat16, tag="h2t_sb")

    # Input DMAs: large contiguous-per-partition transfers, ordered by first
    # use: the x(+leads) prefix unblocks mi=0, the rest streams behind while
    # the first h-tiles compute. All inputs stay on the Sync HWDGE ring:
    # routing pieces through GpSimd SWDGE was tried and is slower (Q7
    # descriptor generation) and once crashed the device.
    nc.sync.dma_start(out=x_sb[:], in_=dram["xt"])
    a = 0
    if head:
        # Dedicated packed lead tensors: contiguous per partition, so the
        # lead can be a single h-tile (0.25MB) — smallest possible prefix.
        nc.sync.dma_start(out=w3_sb[:, :, :LEADW], in_=dram["w3l"])
        nc.sync.dma_start(out=w1_sb[:, :, :LEADW], in_=dram["w1l"])
        a = LEADW
    for b in pieces:
        nc.sync.dma_start(out=w3_sb[:, :, a:b], in_=dram["w3"][:, :, a:b])
        nc.sync.dma_start(out=w1_sb[:, :, a:b], in_=dram["w1"][:, :, a:b])
        a = b
    nc.sync.dma_start(out=w2_sb[:], in_=dram["w2"])

    # Phase A: h2T[H, Ttok] = (W1^T x^T) * silu(W3^T x^T), bf16.
    for mi in range(KH):
        hsl = slice(mi * P, (mi + 1) * P)
        for o, nw in chunks:
            # ps3 accumulates FIRST: its sigmoid+mul evict then overlaps ps1's
            # matmuls, leaving only the final h2t mul exposed after ps1 stops.
            ps3 = pools["psA"].tile([P, 512], mybir.dt.float32, tag="ps3", bufs=2)
            ps1 = pools["psA"].tile([P, 512], mybir.dt.float32, tag="ps1", bufs=2)
            for k in range(KD):
                nc.tensor.matmul(
                    ps3[:, :nw],
                    lhsT=w3_sb[:, k : k + 1, hsl],
                    rhs=x_sb[:, k : k + 1, o : o + nw],
                    start=(k == 0),
                    stop=(k == KD - 1),
                )
            for k in range(KD):
                nc.tensor.matmul(
                    ps1[:, :nw],
                    lhsT=w1_sb[:, k : k + 1, hsl],
                    rhs=x_sb[:, k : k + 1, o : o + nw],
                    start=(k == 0),
                    stop=(k == KD - 1),
                )
            # silu = h3 * sigmoid(h3); split sigmoid+mul beats the ACT Silu
            # table on HW (cold-table cost) and matches CoreSim.
            sig = pools["tmp"].tile([P, 512], mybir.dt.float32, tag="sig", bufs=2)
            nc.scalar.activation(
                sig[:, :nw], ps3[:, :nw], mybir.ActivationFunctionType.Sigmoid
            )
            # silu in place: sig <- ps3 * sig (same-engine reuse, fewer tiles)
            nc.vector.tensor_mul(sig[:, :nw], ps3[:, :nw], sig[:, :nw])
            nc.vector.tensor_mul(h2t_sb[:, mi, o : o + nw], ps1[:, :nw], sig[:, :nw])

    # Phase B: yT[D, Ttok] = W2^T @ h2T. Tokens are the moving dim, so the
    # ragged token count costs proportionally; each D-tile's output row block
    # is contiguous in DRAM (one fast output DMA per tile).
    for di in range(D // P):
        dsl = slice(di * P, (di + 1) * P)
        # bufs=4: pass 1's outputs ride the slow-draining Scalar ring
        # (~25-30 GB/s, ~5.1us per dtile vs the 3.4us compute cadence), so a
        # 4-deep slot pipeline absorbs the queue lag without stalling the
        # phase-B evictions.
        yt = pools["yt"].tile([P, Ttok], mybir.dt.bfloat16, tag="yt", bufs=4)
        for o, nw in chunks:
            ps = pools["psB"].tile([P, 512], mybir.dt.float32, tag="psB", bufs=3)
            for k in range(KH):
                nc.tensor.matmul(
                    ps[:, :nw],
                    lhsT=w2_sb[:, k : k + 1, dsl],
                    rhs=h2t_sb[:, k : k + 1, o : o + nw],
                    start=(k == 0),
                    stop=(k == KH - 1),
                )
            nc.vector.tensor_copy(out=yt[:, o : o + nw], in_=ps[:, :nw])
            # Per-chunk output tensors (when provided) are each contiguous
            # in DRAM, so chunk outputs ship as soon as they're evicted and
            # only a half-size transfer (+HBM write receipt) trails the last
            # matmul. (A column slice of one [D,C] tensor would be a strided
            # write — the measured slow DMA pattern.)
            if "y_parts" in dram:
                ci = next(i for i, (po, _) in enumerate(chunks) if po == o)
                getattr(nc, out_eng).dma_start(
                    out=dram["y_parts"][ci][dsl, :], in_=yt[:, o : o + nw]
                )
        # Output engine choice (measured): pass 1 outputs ride the Scalar
        # HWDGE ring — it drains nearly serially (~25-30 GB/s) but pass-1
        # outputs have ~80us of slack, and avoiding the GpSimd SWDGE ring
        # entirely saves ~3-6us of SWDGE dma_reset in the exit barrier
        # butterfly (no-SWDGE runs measured a 2.0us butterfly vs 5.3-8.5us
        # with it). Pass 2 outputs ride the Sync HWDGE ring, which is empty
        # once the input stream is done and drains in parallel — keeping the
        # final output off the critical tail.
        if "y_parts" not in dram:
            getattr(nc, out_eng).dma_start(out=dram["y"][dsl, :], in_=yt[:])


def _build_nc(C, SS):
    """Build the per-core Bass program: shared FFN ([SS] tokens) + routed FFN ([C])."""
    nc = bacc.Bacc("TRN2", target_bir_lowering=False, debug=False)

    bf = mybir.dt.bfloat16
    shared = {
        "xt": nc.dram_tensor("xst", [P, KD, SS], bf, kind="ExternalInput").ap(),
        "w1": nc.dram_tensor("ws1", [P, KD, H], bf, kind="ExternalInput").ap(),
        "w3": nc.dram_tensor("ws3", [P, KD, H], bf, kind="ExternalInput").ap(),
        "w1l": nc.dram_tensor("ws1l", [P, KD, LEADW], bf, kind="ExternalInput").ap(),
        "w3l": nc.dram_tensor("ws3l", [P, KD, LEADW], bf, kind="ExternalInput").ap(),
        "w2": nc.dram_tensor("ws2", [P, KH, D], bf, kind="ExternalInput").ap(),
        "y": nc.dram_tensor("yst", [D, SS], bf, kind="ExternalOutput").ap(),
    }
    routed = {
        "xt": nc.dram_tensor("xgt", [P, KD, C], bf, kind="ExternalInput").ap(),
        "w1": nc.dram_tensor("w1", [P, KD, H], bf, kind="ExternalInput").ap(),
        "w3": nc.dram_tensor("w3", [P, KD, H], bf, kind="ExternalInput").ap(),
        "w2": nc.dram_tensor("w2", [P, KH, D], bf, kind="ExternalInput").ap(),
        "y_parts": [
            nc.dram_tensor(f"ygt{i}", [D, nw], bf, kind="ExternalOutput").ap()
            for i, (_, nw) in enumerate(_equal_chunks(C))
        ],
    }

    with tile.TileContext(nc) as tc, ExitStack() as ctx:
        pools = {
            "x": ctx.enter_context(tc.tile_pool(name="x", bufs=1)),
            "xg": ctx.enter_context(tc.tile_pool(name="xg", bufs=1)),
            "wA": ctx.enter_context(tc.tile_pool(name="wA", bufs=1)),
            "wB": ctx.enter_context(tc.tile_pool(name="wB", bufs=1)),
            "h2t": ctx.enter_context(tc.tile_pool(name="h2t", bufs=1)),
            "tmp": ctx.enter_context(tc.tile_pool(name="tmp", bufs=2)),
            "yt": ctx.enter_context(tc.tile_pool(name="yt", bufs=2)),
            "psA": ctx.enter_context(tc.tile_pool(name="psA", bufs=2, space="PSUM")),
            "psB": ctx.enter_context(tc.tile_pool(name="psB", bufs=3, space="PSUM")),
        }
        # HAM warm-up: cold matmuls on a zeroed tile while the input DMAs
        # stream in, so the PE clock-gate is at 8/8 when real work starts.
        warm = pools["tmp"].tile([P, P], mybir.dt.bfloat16, tag="warm", bufs=1)
        nc.vector.memset(warm[:], 0.0)
        wps = pools["psB"].tile([P, P], mybir.dt.float32, tag="psB", name="wps")
        for _ in range(WARMUP_MM):
            nc.tensor.matmul(wps[:], lhsT=warm[:], rhs=warm[:], start=True, stop=True)
        # Shared pass first: its 1MB x block + weight leads unblock the PE
        # soonest. The routed pass's x prefetches early into its own slot
        # ("xg" pool); its weights stream into the freed "wA"/"wB" slots
        # during the shared pass's phase B.
        sh_pools = dict(pools)
        _emit_pass(tc, sh_pools, shared, SS, PIECES_FINE, head=True)
        ro_pools = dict(pools)
        ro_pools["x"] = pools["xg"]
        _emit_pass(tc, ro_pools, routed, C, PIECES_COARSE, out_eng="sync")

    nc.compile()
    return nc


def _route(x, Wr, rb):
    """Replicate the reference router. Returns (idx [T,2] int, w [T,2] f32).

    Uses jax on CPU with the exact expressions from the reference so the top-2
    selection bit-matches a CPU-run reference (min 2nd-vs-3rd logit gap in this
    problem is ~1e-6, so the selection must match the reference's fp32 math).
    Falls back to numpy float64 if jax-cpu is unavailable.
    """
    try:
        import jax
        import jax.numpy as jnp

        cpu = jax.devices("cpu")[0]
        with jax.default_device(cpu):
            xl = jnp.asarray(np.asarray(x))
            wr = jnp.asarray(np.asarray(Wr))
            rbj = jnp.asarray(np.asarray(rb))
            logits = jnp.einsum("bsd,de->bse", xl, wr) * SCALE
            _, idx = jax.lax.top_k(logits + rbj, TOPK)
            gathered = jnp.take_along_axis(logits, idx, axis=-1)
            w = jax.nn.softmax(gathered, axis=-1)
        idx = np.asarray(idx).reshape(-1, TOPK)
        w = np.asarray(w, dtype=np.float32).reshape(-1, TOPK)
        return idx, w
    except Exception:
        xf = np.asarray(x, np.float64).reshape(-1, D)
        logits = (xf @ np.asarray(Wr, np.float64)) * SCALE
        biased = logits + np.asarray(rb, np.float64)
        idx = np.argsort(-biased, axis=-1)[:, :TOPK]
        g = np.take_along_axis(logits, idx, axis=-1)
        g = g - g.max(axis=-1, keepdims=True)
        wexp = np.exp(g)
        w = (wexp / wexp.sum(axis=-1, keepdims=True)).astype(np.float32)
        return idx, w


def _pack_p(a2d, kchunks):
    """[R, cols] -> p-major [128, kchunks, cols] (R = kchunks*128)."""
    cols = a2d.shape[1]
    return np.ascontiguousarray(
        a2d.reshape(kchunks, P, cols).transpose(1, 0, 2)
    )


def kernel(x, Wr, rb, W1, W2, W3, Ws1, Ws2, Ws3):
    global LAST_RESULTS
    x = np.asarray(x, np.float32)
    Wr = np.asarray(Wr, np.float32)
    rb = np.asarray(rb, np.float32)
    W1 = np.asarray(W1, np.float32)
    W2 = np.asarray(W2, np.float32)
    W3 = np.asarray(W3, np.float32)
    Ws1 = np.asarray(Ws1, np.float32)
    Ws2 = np.asarray(Ws2, np.float32)
    Ws3 = np.asarray(Ws3, np.float32)

    T = B * S
    xf = x.reshape(T, D)

    # ---- Router (host, exact) ----
    idx, w = _route(x, Wr, rb)

    # ---- Shard ----
    toks = [np.nonzero((idx == e).any(axis=1))[0] for e in range(E)]
    wtok = [
        w[toks[e], :][idx[toks[e], :] == e].astype(np.float32) for e in range(E)
    ]
    counts = [len(t) for t in toks]
    # Cap each expert's device block at the balanced mean (T*K/E = 1024); the
    # few overflow tokens of over-subscribed experts (~1.5% of routed work)
    # are computed on the host in fp32. Every core then does exactly the mean
    # routed load instead of all cores padding to the max expert's count, and
    # the token chunks split into even 512s.
    CAP = T * TOPK // E
    C = max(256, min(max(counts), CAP))
    dev_counts = [min(c, C) for c in counts]
    SS = T // NCORES

    xf_bf = xf.astype(BF16)
    ws1p = _pack_p(Ws1.astype(BF16), KD)
    ws3p = _pack_p(Ws3.astype(BF16), KD)
    ws2p = _pack_p(Ws2.astype(BF16), KH)
    ws1l = np.ascontiguousarray(ws1p[:, :, :LEADW])
    ws3l = np.ascontiguousarray(ws3p[:, :, :LEADW])
    in_maps = []
    for e in range(E):
        xg = np.zeros((C, D), dtype=BF16)
        xg[: dev_counts[e]] = xf_bf[toks[e][: dev_counts[e]]]
        in_maps.append(
            {
                "xst": _pack_p(
                    np.ascontiguousarray(xf_bf[e * SS : (e + 1) * SS].T), KD
                ),
                "ws1": ws1p,
                "ws3": ws3p,
                "ws1l": ws1l,
                "ws3l": ws3l,
                "ws2": ws2p,
                "xgt": _pack_p(np.ascontiguousarray(xg.T), KD),
                "w1": _pack_p(W1[e].astype(BF16), KD),
                "w3": _pack_p(W3[e].astype(BF16), KD),
                "w2": _pack_p(W2[e].astype(BF16), KH),
            }
        )

    # ---- Device ----
    key = (C, SS)
    if key not in _NC_CACHE:
        _NC_CACHE[key] = _build_nc(C, SS)
    nc = _NC_CACHE[key]
    res = run_bass_kernel_spmd(nc, in_maps, list(range(NCORES)))
    LAST_RESULTS = res

    # ---- Combine (host) ----
    out = np.empty((T, D), dtype=np.float32)
    for e in range(E):
        out[e * SS : (e + 1) * SS] = res.results[e]["yst"].astype(np.float32).T
    nyg = len(_equal_chunks(C))
    for e in range(E):
        yg = np.concatenate(
            [res.results[e][f"ygt{i}"].astype(np.float32).T for i in range(nyg)],
            axis=0,
        )[: dev_counts[e]]
        out[toks[e][: dev_counts[e]]] += wtok[e][: dev_counts[e], None] * yg
        if dev_counts[e] < counts[e]:
            # fp32 host FFN for the capacity-overflow tokens of this expert
            ot = toks[e][dev_counts[e] :]
            xe = xf[ot]
            h3 = xe @ W3[e]
            h = (xe @ W1[e]) * (h3 / (1.0 + np.exp(-h3)))
            out[ot] += wtok[e][dev_counts[e] :, None] * (h @ W2[e])
    return out.reshape(B, S, D)
